# revision 18
# baseline (speedup 1.0000x reference)
"""Trainium2 Bass kernel for nn_AutoRegressive_12128987644588.

6-layer post-norm transformer decoder (self-attn w/ prefix-causal mask,
cross-attn to packed embeddings, FFN), B=4, seq 865 (pad 896), D=1024,
16 heads x 64, FF=4096, final proj to 1024.

Sharding: 8 cores = 4 batches x 2 sequence halves of 448 tokens.
Per layer the two cores of a batch AllGather their x^T halves (the only
collective); K/V projections are computed over the full sequence on both
cores (duplicate compute, no other comm). Activations live transposed
[feature, token] in SBUF so every GEMM is natural (lhsT = W^T chunk,
rhs = x^T chunk) and all out-feature biases are per-partition. x tiles
are updated in place (residual adds and LayerNorm write back).

v2 performance structure (vs the v1 baseline):
- Attention exps are batched: scores for 2 key tiles of a head land in
  one 2-bank PSUM tile (slices at 0/512) and a single wide ACTIVATE
  covers both, amortizing the ~352-cycle ACT pipeline fill.
- No ACT table switches: LayerNorm rstd = exp(-0.5*ln(var+eps)) so the
  whole kernel lives in the natural_log_exp set (exp/ln/relu/square).
- All small stationary operands (ones rows, k2sel, LN w/b rows) are
  fp16 so their matmuls are single-pass (f32 stationaries run 2-pass
  HIGH/LOW).
- memT (cross-attn memory) is loaded to SBUF once and kept resident.
- Attention/FFN weights use separate SBUF pools so next-phase weight
  DMAs prefetch while the other phase's weights cycle.
- CA K/V projection is emitted with a 1-bank PSUM pool and late
  priority so the Tile scheduler drops its matmuls into the PE gaps of
  the (ACT-bound) self-attention phase and the AllGather window.
- Invalid-key masking for CA uses zeroed aug-ones rows in V (no exp
  bias), structurally zero biases are skipped (build flags).

Embedding gather/pack/positional encodings are integer-indexed data
staging done on host; all FLOPs run on device.
"""
import numpy as np

import concourse.bass as bass
import concourse.mybir as mybir
import concourse.tile as tile
from concourse import bacc, bass_utils

F32 = mybir.dt.float32
F32R = mybir.dt.float32r
BF16 = mybir.dt.float16  # fp16: FWL-eligible, 10-bit mantissa

B, D, H, HD, FF, L = 4, 1024, 16, 64, 4096, 6
TT, TA, ENR = 128, 512, 225
SEQ = TT + TA + ENR            # 865
TPAD = 896                     # 7 * 128
TH = 448                       # per-core half (padded)
PREFIX = TT + TA               # 640 = 5 * 128
NKT = TPAD // 128              # 7 key tiles
ND = D // 128                  # 8 feature tiles
VOCAB = 1024
EPS = 1e-5
NEG = -1e9
AF = mybir.ActivationFunctionType
WIDE_EXP = True

# The act-table-load pass maps each ACTIVATE to the first table set
# containing its function, so Exp->exp_and_others and Ln->natural_log
# alternate (a ~2.7us table DMA at every transition). Both functions
# live in natural_log_exp_and_others; restricting Exp/Ln to that one
# set makes the pass settle on it and the kernel runs with a single
# table load. Set indices are untouched, so the emitted ids stay valid.
if not getattr(bacc, "_act_tables_patched", False):
    _orig_get_act_tables = bacc.get_activation_tables

    def _patched_act_tables(arch):
        tabs = _orig_get_act_tables(arch)
        combined = "natural_log_exp_and_others"
        if combined in tabs and AF.Exp in tabs[combined]:
            for name, s in tabs.items():
                if name != combined:
                    s.discard(AF.Exp)
                    s.discard(AF.Ln)
        return tabs

    bacc.get_activation_tables = _patched_act_tables
    bacc._act_tables_patched = True


# ---------------------------------------------------------------- host side

def sinusoidal_pe(T, d):
    pos = np.arange(T, dtype=np.float32)[:, None]
    div = np.exp(np.arange(0, d, 2, dtype=np.float32) * (-np.log(10000.0) / d))
    pe = np.zeros((T, d), dtype=np.float32)
    pe[:, 0::2] = np.sin(pos * div)
    pe[:, 1::2] = np.cos(pos * div)
    return pe


def host_embed(text, audio, enrolled_audio, text_len, audio_len,
               text_emb, audio_emb):
    """Replicates reference embed+pack. Returns [B, TPAD, D] f32 (pad zeros)."""
    te = text_emb[text] + sinusoidal_pe(TT, D)[None]        # [B,TT,D]
    ae = audio_emb[audio] + sinusoidal_pe(TA, D)[None]      # [B,TA,D]
    ee = audio_emb[enrolled_audio] + sinusoidal_pe(ENR, D)[None]
    out = np.zeros((B, TPAD, D), dtype=np.float32)
    for b in range(B):
        tl, al = int(text_len[b]), int(audio_len[b])
        out[b, :tl] = te[b, :tl]
        out[b, tl:tl + al] = ae[b, :al]
        out[b, tl + al:tl + al + ENR] = ee[b]
    return out


def host_masks(half):
    """Additive mask for SA key tiles 5,6 transposed: [256, TH]."""
    k = np.arange(PREFIX, PREFIX + 256)[:, None]            # 640..895
    q = half * TH + np.arange(TH)[None, :]
    blocked = (k > q) | (k >= SEQ)
    return np.where(blocked, NEG, 0.0).astype(np.float32)


def host_kvalid6():
    k = PREFIX + 128 + np.arange(128)                       # 768..895
    return np.where(k < SEQ, 0.0, NEG).astype(np.float32)[:, None]


# ---------------------------------------------------------------- builder

def build_kernel(n_layers=L, skip_bv=False, skip_inb=False, skip_outb=False,
                 skip_ffb=False, skip_yb=False):
    nc = bacc.Bacc("TRN2", target_bir_lowering=False, debug=False,
                   num_devices=8)

    def din(name, shape, dt=F32R):
        return nc.dram_tensor(name, shape, dt, kind="ExternalInput")

    xT0_d = din("xT0", [D, TH])
    memT_d = din("memT", [D, TPAD], BF16)
    maskT_d = din("maskT", [256, TH], BF16)
    kval6_d = din("kval6", [128, 1], F32)
    oodc_d = din("oodc", [128, 1])            # 1/D column, f32r
    oodc16_d = din("oodc16", [128, 1], BF16)
    ones_r128_d = din("ones_r128", [1, 128])  # f32r ones row (bv path)
    onesb_r128_d = din("onesb_r128", [1, 128], BF16)
    k2sel_d = din("k2sel", [2, 128], BF16)
    negb_r448_d = din("negb_r448", [1, TH], BF16)

    sa_inT_d = din("sa_inT", [L, D, 3 * D], BF16)
    sa_outT_d = din("sa_outT", [L, D, D], BF16)
    ca_inT_d = din("ca_inT", [L, D, 3 * D], BF16)
    ca_outT_d = din("ca_outT", [L, D, D], BF16)
    ff1T_d = din("ff1T", [L, D, FF], BF16)
    ff2T_d = din("ff2T", [L, FF, D], BF16)
    outT_d = din("outT", [D, VOCAB], BF16)

    sa_inb_d = din("sa_inb", [L, 3 * D], F32)
    sa_outb_d = din("sa_outb", [L, D], F32)
    ca_inb_d = din("ca_inb", [L, 3 * D], F32)
    ca_outb_d = din("ca_outb", [L, D], F32)
    ff1b_d = din("ff1b", [L, FF], F32)
    ff2b_d = din("ff2b", [L, D], F32)
    outb_d = din("outb", [VOCAB], F32)
    lnwc_d = din("lnwc", [128, 3 * L * (D // 128)], F32)
    lnwb16_d = din("lnwb16", [3, L, 2, D], BF16)  # [ln idx][layer][w;b][D]

    yT_d = nc.dram_tensor("yT", [VOCAB, TH], F32, kind="ExternalOutput")

    uid = [0]

    def nm(p):
        uid[0] += 1
        return f"{p}_{uid[0]}"

    with tile.TileContext(nc) as tc:
        with (
            nc.allow_low_precision(reason="f32r/bf16 compute; tol 2e-2"),
            tc.tile_pool(name="const", bufs=1) as constp,
            tc.tile_pool(name="memp", bufs=1) as memp,
            tc.tile_pool(name="xpool", bufs=8) as xpool,
            tc.tile_pool(name="tmpp", bufs=3) as tmpp,
            tc.tile_pool(name="rows", bufs=2) as rowp,
            tc.tile_pool(name="statp", bufs=4) as statp,
            tc.tile_pool(name="biasp", bufs=10) as biasp,
            tc.tile_pool(name="attw", bufs=14) as attw,
            tc.tile_pool(name="ffw", bufs=10) as ffw,
            tc.tile_pool(name="attsb", bufs=2) as attsb,
            tc.tile_pool(name="dram", bufs=2, space="DRAM") as dramp,
        ):
            # ---- constants
            oodc = constp.tile([128, 1], F32R, name="oodc")
            oodc16 = constp.tile([128, 1], BF16, name="oodc16")
            ones_r128 = constp.tile([1, 128], F32R, name="ones_r128")
            onesb_r128 = constp.tile([1, 128], BF16, name="onesb_r128")
            kval6 = constp.tile([128, 1], F32, name="kval6")
            # wide additive mask for SA windows (5,6): slices at 0 / 512
            maskw56 = constp.tile([128, 1024], BF16, name="maskw56")
            k2sel_a = constp.tile([1, 128], BF16, name="k2sel_a")
            k2sel_b = constp.tile([1, 128], BF16, name="k2sel_b")
            lnrhs = constp.tile([2, TH], BF16, name="lnrhs")
            lnwc = constp.tile([128, 3 * L * ND], F32, name="lnwc")
            eps_tile = constp.tile([1, 1], F32, name="eps_tile")
            nc.sync.dma_start(out=oodc[:], in_=oodc_d.ap())
            nc.sync.dma_start(out=oodc16[:], in_=oodc16_d.ap())
            nc.sync.dma_start(out=ones_r128[:], in_=ones_r128_d.ap())
            nc.sync.dma_start(out=onesb_r128[:], in_=onesb_r128_d.ap())
            nc.sync.dma_start(out=kval6[:], in_=kval6_d.ap())
            nc.sync.dma_start(out=maskw56[:, 0:TH], in_=maskT_d.ap()[0:128, :])
            nc.sync.dma_start(out=maskw56[:, 512:512 + TH],
                              in_=maskT_d.ap()[128:256, :])
            nc.sync.dma_start(out=k2sel_a[:], in_=k2sel_d.ap()[0:1, :])
            nc.sync.dma_start(out=k2sel_b[:], in_=k2sel_d.ap()[1:2, :])
            nc.sync.dma_start(out=lnrhs[1:2, :], in_=negb_r448_d.ap())
            nc.sync.dma_start(out=lnwc[:], in_=lnwc_d.ap())
            nc.vector.memset(eps_tile[:], EPS)
            nc.vector.memset(maskw56[:, TH:512], 0.0)

            # ---- x tiles (f32r master) + bf16 shadows for GEMM moving ops
            x_cur = []
            xb16 = []
            for t in range(ND):
                xt = xpool.tile([128, TH], F32R, name=nm("x"), tag="x")
                nc.sync.dma_start(out=xt[:],
                                  in_=xT0_d.ap()[t * 128:(t + 1) * 128, :])
                x_cur.append(xt)
                xb = xpool.tile([128, TH], BF16, name=nm("xb"), tag="xb")
                nc.vector.tensor_copy(xb[:], xt[:])
                xb16.append(xb)

            # ---- persistent cross-attention memory (layer-invariant)
            memt = []
            for t in range(ND):
                mt = memp.tile([128, TPAD], BF16, name=nm("memt"), tag="mem",
                               bufs=8)
                nc.sync.dma_start(
                    out=mt[:], in_=memT_d.ap()[t * 128:(t + 1) * 128, :])
                memt.append(mt)

            # ---------------------------------------------------- helpers
            def load_bias_col(src_1d_ap, n, name):
                t = biasp.tile([128, n], F32, name=nm(name), tag="bcol")
                nc.sync.dma_start(
                    out=t[:], in_=src_1d_ap.rearrange("(c p) -> p c", p=128))
                return t

            def load_row(src_1d_ap, n, name):
                t = rowp.tile([1, n], F32R, name=nm(name), tag="row")
                nc.sync.dma_start(
                    out=t[:],
                    in_=src_1d_ap.rearrange("(a f) -> a f", a=1).bitcast(F32R))
                return t

            def proj_gemm(wT2d, rhs_tiles, nout, wpool, wtag, wbufs, evict,
                          fdim=TH, pbufs=3):
                """out^T[nout, fdim] = W @ rhs. evict(n0, psum) per 128 rows."""
                nk = len(rhs_tiles)
                ctx = tc.tile_pool(name=nm("gps"), bufs=pbufs, space="PSUM")
                ppool = ctx.__enter__()
                for n0 in range(0, nout, 512):
                    w = min(512, nout - n0)
                    wts = []
                    for k in range(nk):
                        wt = wpool.tile([128, w], BF16, name=nm("w"), tag=wtag,
                                        bufs=wbufs)
                        nc.sync.dma_start(
                            out=wt[:],
                            in_=wT2d[k * 128:(k + 1) * 128, n0:n0 + w])
                        wts.append(wt)
                    for m0 in range(0, w, 128):
                        ps = ppool.tile([128, fdim], F32, name=nm("pg"),
                                        tag="pg", bufs=pbufs)
                        for k in range(nk):
                            nc.tensor.matmul(
                                ps[:], wts[k][:, m0:m0 + 128],
                                rhs_tiles[k][:, :fdim],
                                start=(k == 0), stop=(k == nk - 1))
                        evict(n0 + m0, ps)
                ctx.__exit__(None, None, None)

            def layer_norm(x_tiles, w_col, w_row):
                """In-place post-norm LN over the feature (partition) dim.
                rstd = exp(-0.5*ln(var+eps)) keeps ACT in the ln/exp table
                set; stats stay in PSUM and feed DVE directly."""
                with tc.tile_pool(name=nm("lnps"), bufs=1, space="PSUM") as lps:
                    mu_ps = lps.tile([1, TH], F32, name=nm("mups"), bufs=1)
                    s2_ps = lps.tile([1, TH], F32, name=nm("s2ps"), bufs=1)
                    for t in range(ND):
                        nc.tensor.matmul(mu_ps[:], oodc[:], x_tiles[t][:],
                                         start=(t == 0), stop=(t == ND - 1))
                    for t in range(ND):
                        sq = tmpp.tile([128, TH], BF16, name=nm("sq"),
                                       tag="sq16", bufs=3)
                        nc.vector.tensor_tensor(sq[:], x_tiles[t][:],
                                                x_tiles[t][:],
                                                mybir.AluOpType.mult)
                        nc.tensor.matmul(s2_ps[:], oodc16[:], sq[:],
                                         start=(t == 0), stop=(t == ND - 1))
                    mu_sb = statp.tile([1, TH], F32, name=nm("musb"),
                                       tag="st")
                    nc.vector.tensor_copy(mu_sb[:], mu_ps[:])
                    musq = statp.tile([1, TH], F32, name=nm("musq"),
                                      tag="st")
                    nc.vector.tensor_tensor(musq[:], mu_sb[:], mu_sb[:],
                                            mybir.AluOpType.mult)
                    var = statp.tile([1, TH], F32, name=nm("var"), tag="st")
                    nc.vector.tensor_tensor(var[:], s2_ps[:], musq[:],
                                            mybir.AluOpType.subtract)
                    lnv = statp.tile([1, TH], F32, name=nm("lnv"), tag="st")
                    nc.scalar.activation(lnv[:], var[:], AF.Ln,
                                         bias=eps_tile[:])
                    rstd = statp.tile([1, TH], BF16, name=nm("rstd"),
                                      tag="st")
                    nc.scalar.activation(rstd[:], lnv[:], AF.Exp, scale=-0.5)
                    rsb_ps = lps.tile([128, TH], F32, name=nm("rsb"), bufs=1)
                    nc.tensor.matmul(rsb_ps[:], onesb_r128[:], rstd[:],
                                     start=True, stop=True)
                    nc.vector.tensor_tensor(lnrhs[0:1, :], mu_sb[:], rstd[:],
                                            mybir.AluOpType.mult)
                    for t in range(ND):
                        aux = lps.tile([128, TH], F32, name=nm("aux"),
                                       tag="lnaux", bufs=2)
                        nc.tensor.matmul(aux[:],
                                         w_row[:, t * 128:(t + 1) * 128],
                                         lnrhs[:], start=True, stop=True)
                        t1 = tmpp.tile([128, TH], F32R, name=nm("t1"),
                                       tag="tmp")
                        nc.vector.tensor_tensor(t1[:], x_tiles[t][:],
                                                rsb_ps[:],
                                                mybir.AluOpType.mult)
                        nc.vector.scalar_tensor_tensor(
                            x_tiles[t][:], t1[:], w_col[:, t:t + 1], aux[:],
                            mybir.AluOpType.mult, mybir.AluOpType.subtract)
                        nc.scalar.copy(xb16[t][:], x_tiles[t][:])

            def attention(q_tiles, kt_tiles, vaug_tiles, masked, kval):
                """Returns attnT tiles (8 x [128, TH]).
                Heads processed singly; per head the 7 key tiles are
                grouped into windows of 2 sharing one 2-bank PSUM tile so
                a single wide ACTIVATE computes both exps. SA groups the
                masked tiles (5,6) together and adds maskw56 in one wide
                DVE op. PSUM: swide 2x2 + o_ps 2 + rps 1 + filler 1 = 8."""
                at = [attsb.tile([128, TH], BF16, name=nm("at"), tag="attnT",
                                 bufs=8) for _ in range(ND)]
                if masked:
                    windows = [(0, 1), (2, 3), (4,), (5, 6)]
                else:
                    windows = [(0, 1), (2, 3), (4, 5), (6,)]
                with (
                    tc.tile_pool(name=nm("aps"), bufs=2, space="PSUM") as sps,
                    tc.tile_pool(name=nm("ops"), bufs=2, space="PSUM") as ops,
                    tc.tile_pool(name=nm("bps"), bufs=1, space="PSUM") as bps,
                ):
                    o_ps = {}
                    for hh in range(H):
                        ti, r0 = hh // 2, (hh % 2) * 64
                        o_ps[hh] = ops.tile([65, TH], F32, name=nm("ops"),
                                            tag="po", bufs=2)
                        for win in windows:
                            sw = sps.tile([128, 1024], F32, name=nm("sw"),
                                          tag="sw", bufs=2)
                            if WIDE_EXP and len(win) == 2:
                                nc.vector.memset(sw[:, TH:512], 0.0)
                            for j, t in enumerate(win):
                                nc.tensor.matmul(
                                    sw[:, j * 512:j * 512 + TH],
                                    kt_tiles[ti][r0:r0 + 64,
                                                 t * 128:(t + 1) * 128],
                                    q_tiles[ti][r0:r0 + 64, :],
                                    start=True, stop=True)
                            wd = (win[-1] - win[0]) * 512 + TH
                            pbw = attsb.tile([128, 1024], BF16, name=nm("p"),
                                             tag="pexp", bufs=3)
                            if WIDE_EXP:
                                if masked and win[0] == 5:
                                    sm = attsb.tile([128, 1024], BF16,
                                                    name=nm("sm"), tag="pexp",
                                                    bufs=3)
                                    nc.vector.tensor_tensor(
                                        sm[:, 0:wd], sw[:, 0:wd],
                                        maskw56[:, 0:wd],
                                        mybir.AluOpType.add)
                                    nc.scalar.activation(pbw[:, 0:wd],
                                                         sm[:, 0:wd], AF.Exp)
                                elif (kval is not None
                                      and win[-1] == NKT - 1):
                                    nc.scalar.activation(pbw[:, 0:wd],
                                                         sw[:, 0:wd], AF.Exp,
                                                         bias=kval[:])
                                else:
                                    nc.scalar.activation(pbw[:, 0:wd],
                                                         sw[:, 0:wd], AF.Exp)
                            else:
                                for j, t in enumerate(win):
                                    c0 = j * 512
                                    if masked and win[0] == 5:
                                        sm = attsb.tile(
                                            [128, 1024], BF16,
                                            name=nm("sm"), tag="pexp",
                                            bufs=3)
                                        nc.vector.tensor_tensor(
                                            sm[:, c0:c0 + TH],
                                            sw[:, c0:c0 + TH],
                                            maskw56[:, c0:c0 + TH],
                                            mybir.AluOpType.add)
                                        nc.scalar.activation(
                                            pbw[:, c0:c0 + TH],
                                            sm[:, c0:c0 + TH], AF.Exp)
                                    elif (kval is not None
                                          and t == NKT - 1):
                                        nc.scalar.activation(
                                            pbw[:, c0:c0 + TH],
                                            sw[:, c0:c0 + TH], AF.Exp,
                                            bias=kval[:])
                                    else:
                                        nc.scalar.activation(
                                            pbw[:, c0:c0 + TH],
                                            sw[:, c0:c0 + TH], AF.Exp)
                            for j, t in enumerate(win):
                                nc.tensor.matmul(
                                    o_ps[hh][:],
                                    vaug_tiles[t][:].rearrange(
                                        "p (h e) -> p h e", e=65)[:, hh, :],
                                    pbw[:, j * 512:j * 512 + TH],
                                    start=(t == 0), stop=(t == NKT - 1))
                        if hh % 2 == 1:
                            hp = hh - 1
                            r_ps = bps.tile([128, TH], F32, name=nm("rps"),
                                            tag="pb", bufs=1)
                            for h2 in (hp, hp + 1):
                                den = statp.tile([1, TH], BF16,
                                                 name=nm("den"), tag="st")
                                nc.vector.tensor_copy(den[:],
                                                      o_ps[h2][64:65, :])
                                nc.tensor.matmul(r_ps[:],
                                                 (k2sel_a if h2 % 2 == 0
                                                  else k2sel_b)[:],
                                                 den[:],
                                                 start=(h2 % 2 == 0),
                                                 stop=(h2 % 2 == 1))
                            rb = tmpp.tile([128, TH], F32, name=nm("rb"),
                                           tag="rb", bufs=2)
                            nc.vector.reciprocal_approx_fast(out=rb[:],
                                                             in_=r_ps[:])
                            for h2 in (hp, hp + 1):
                                ti2, r2 = h2 // 2, (h2 % 2) * 64
                                nc.vector.tensor_tensor(
                                    at[ti2][r2:r2 + 64, :],
                                    o_ps[h2][0:64, :],
                                    rb[r2:r2 + 64, :],
                                    mybir.AluOpType.mult)
                return at

            def kv_gemm(tag, inT2d, inb1d, src_tiles, vones_t6, psum_pool,
                        pbufs):
                """K^T tiles [8 x (128, TPAD)] + V_aug [7 x (128, H*65)].
                K weights are loaded once and reused for both halves."""
                kt = [attsb.tile([128, TPAD], BF16, name=nm("kt"),
                                 tag=tag + "k", bufs=8) for _ in range(ND)]
                bk_col = None if skip_inb else load_bias_col(
                    inb1d[D:2 * D], ND, "bk")
                for c0 in (0, 512):
                    wts = []
                    for k in range(ND):
                        wt = attw.tile([128, 512], BF16, name=nm("wk"),
                                       tag="aw", bufs=14)
                        nc.sync.dma_start(
                            out=wt[:],
                            in_=inT2d[k * 128:(k + 1) * 128,
                                      D + c0:D + c0 + 512])
                        wts.append(wt)
                    for f0 in (0, TH):
                        for m0 in range(0, 512, 128):
                            ps = psum_pool.tile([128, TH], F32, name=nm("pk"),
                                                tag="pg", bufs=pbufs)
                            for k in range(ND):
                                nc.tensor.matmul(
                                    ps[:], wts[k][:, m0:m0 + 128],
                                    src_tiles[k][:, f0:f0 + TH],
                                    start=(k == 0), stop=(k == ND - 1))
                            ti = (c0 + m0) // 128
                            if bk_col is None:
                                nc.vector.tensor_copy(
                                    kt[ti][:, f0:f0 + TH], ps[:])
                            else:
                                nc.vector.tensor_scalar_add(
                                    kt[ti][:, f0:f0 + TH], ps,
                                    bk_col[:, ti:ti + 1])
                va = [attsb.tile([128, H * 65], BF16, name=nm("va"),
                                 tag=tag + "v", bufs=NKT) for _ in range(NKT)]
                bv_row = None if skip_bv else load_row(
                    inb1d[2 * D:3 * D], D, "bv")
                for t in range(NKT):
                    aug = va[t][:].rearrange("p (h e) -> p h e",
                                             e=65)[:, :, 64:65]
                    if t == NKT - 1 and vones_t6:
                        nc.vector.memset(aug, 0.0)
                        nc.vector.memset(aug[0:SEQ - PREFIX - 128], 1.0)
                    else:
                        nc.vector.memset(aug, 1.0)
                for c0 in (0, 512):
                    wts = []
                    for k in range(ND):
                        wt = attw.tile([128, 512], BF16, name=nm("wv"),
                                       tag="aw", bufs=14)
                        nc.sync.dma_start(
                            out=wt[:],
                            in_=inT2d[k * 128:(k + 1) * 128,
                                      2 * D + c0:2 * D + c0 + 512])
                        wts.append(wt)
                    for t in range(NKT):
                        ps = psum_pool.tile([128, 512], F32, name=nm("pv"),
                                            tag="pg", bufs=pbufs)
                        for k in range(ND):
                            nc.tensor.matmul(
                                ps[:],
                                src_tiles[k][:, t * 128:(t + 1) * 128],
                                wts[k][:], start=(k == 0),
                                stop=(bv_row is None and k == ND - 1))
                        if bv_row is not None:
                            nc.tensor.matmul(ps[:], ones_r128[:, :128],
                                             bv_row[:, c0:c0 + 512],
                                             start=False, stop=True)
                        nc.vector.tensor_copy(
                            va[t][:].rearrange("p (h e) -> p h e", e=65)
                            [:, c0 // 64:c0 // 64 + 8, 0:64],
                            ps[:].rearrange("p (h e) -> p h e", e=64))
                return kt, va

            def qproj(inT2d, inb1d):
                q_t = [attsb.tile([128, TH], BF16, name=nm("q"), tag="q",
                                  bufs=8) for _ in range(ND)]
                bq_col = None if skip_inb else load_bias_col(
                    inb1d[0:D], ND, "bq")

                def ev_q(n0, ps):
                    ti = n0 // 128
                    if bq_col is None:
                        nc.vector.tensor_copy(q_t[ti][:], ps)
                    else:
                        nc.vector.tensor_scalar_add(
                            q_t[ti][:], ps, bq_col[:, ti:ti + 1])
                proj_gemm(inT2d[:, 0:D], xb16, D, attw, "aw", 14, ev_q, pbufs=2)
                return q_t

            def out_proj(wT2d, b1d, at):
                bo_col = None if skip_outb else load_bias_col(b1d, ND, "bo")

                def ev_o(n0, ps):
                    t = n0 // 128
                    if bo_col is None:
                        nc.vector.tensor_tensor(
                            x_cur[t][:], ps, x_cur[t][:],
                            mybir.AluOpType.add)
                    else:
                        nc.vector.scalar_tensor_tensor(
                            x_cur[t][:], ps, bo_col[:, t:t + 1], x_cur[t][:],
                            mybir.AluOpType.add, mybir.AluOpType.add)
                proj_gemm(wT2d, at, D, attw, "aw", 14, ev_o, pbufs=2)

            def do_ln(idx, l):
                lwb = rowp.tile([2, D], BF16, name=nm(f"ln{idx}wb"),
                                tag="rowb")
                nc.sync.dma_start(out=lwb[:], in_=lnwb16_d.ap()[idx, l])
                b0 = (idx * L + l) * ND
                layer_norm(x_cur, lnwc[:, b0:b0 + ND], lwb)

            # ---------------------------------------------------- layers
            fillp_ctx = tc.tile_pool(name=nm("fillp"), bufs=1, space="PSUM")
            fillp = fillp_ctx.__enter__()
            for l in range(n_layers):
                ag_in = dramp.tile([D, TH], BF16, name=nm("agin"), tag="agi")
                ag_out = dramp.tile([2 * D, TH], BF16, name=nm("agout"),
                                    tag="ago")
                for t in range(ND):
                    nc.sync.dma_start(
                        out=ag_in[t * 128:(t + 1) * 128, :], in_=xb16[t][:])
                nc.gpsimd.collective_compute(
                    "AllGather", mybir.AluOpType.bypass,
                    replica_groups=[[0, 1], [2, 3], [4, 5], [6, 7]],
                    ins=[ag_in[:].opt()], outs=[ag_out[:].opt()])

                # fillp (the CA-K/V 1-bank PSUM pool) was opened near
                # the end of the PREVIOUS layer, before its FFN: the bank
                # is disjoint from the FFN/LN3/attention pools, so the CA
                # K/V matmuls (emitted after the SA attention, i.e. at
                # lower priority) are ready to fill the LN3 chain, the
                # AllGather window and the exp-wait gaps of SA attention.
                q_sa = qproj(sa_inT_d.ap()[l], sa_inb_d.ap()[l])

                # SA side (needs the gathered full sequence)
                xfull = [attsb.tile([128, TPAD], BF16, name=nm("xf"),
                                    tag="xfull", bufs=8) for _ in range(ND)]
                for t in range(ND):
                    nc.sync.dma_start(
                        out=xfull[t][:, 0:TH],
                        in_=ag_out[t * 128:(t + 1) * 128, :])
                    nc.sync.dma_start(
                        out=xfull[t][:, TH:TPAD],
                        in_=ag_out[D + t * 128:D + (t + 1) * 128, :])
                with tc.tile_pool(name=nm("sakvp"), bufs=2,
                                  space="PSUM") as sakvp:
                    kt_sa, va_sa = kv_gemm("sa", sa_inT_d.ap()[l],
                                           sa_inb_d.ap()[l], xfull, False,
                                           sakvp, 2)
                at = attention(q_sa, kt_sa, va_sa, True, None)
                kt_ca, va_ca = kv_gemm("ca", ca_inT_d.ap()[l],
                                       ca_inb_d.ap()[l], memt, skip_bv,
                                       fillp, 1)
                out_proj(sa_outT_d.ap()[l], sa_outb_d.ap()[l], at)
                do_ln(0, l)

                q_ca = qproj(ca_inT_d.ap()[l], ca_inb_d.ap()[l])
                at = attention(q_ca, kt_ca, va_ca, False,
                               None if skip_bv else kval6)
                out_proj(ca_outT_d.ap()[l], ca_outb_d.ap()[l], at)
                do_ln(1, l)

                # ================= FFN =================
                fillp_ctx.__exit__(None, None, None)
                fillp_ctx = tc.tile_pool(name=nm("fillp"), bufs=1,
                                         space="PSUM")
                fillp = fillp_ctx.__enter__()
                ht = [attsb.tile([128, TH], BF16, name=nm("h"), tag="h",
                                 bufs=FF // 128) for _ in range(FF // 128)]
                b1_col = None if skip_ffb else load_bias_col(
                    ff1b_d.ap()[l], FF // 128, "b1")

                def ev_h(n0, ps, ht=ht, b1_col=b1_col):
                    t = n0 // 128
                    if b1_col is None:
                        nc.scalar.activation(ht[t][:], ps, AF.Relu)
                    else:
                        nc.scalar.activation(ht[t][:], ps, AF.Relu,
                                             bias=b1_col[:, t:t + 1])
                proj_gemm(ff1T_d.ap()[l], xb16, FF, ffw, "fw", 10, ev_h,
                          pbufs=4)

                b2_col = None if skip_ffb else load_bias_col(
                    ff2b_d.ap()[l], ND, "b2")

                def ev_f(n0, ps, b2_col=b2_col):
                    t = n0 // 128
                    if b2_col is None:
                        nc.vector.tensor_tensor(
                            x_cur[t][:], ps, x_cur[t][:],
                            mybir.AluOpType.add)
                    else:
                        nc.vector.scalar_tensor_tensor(
                            x_cur[t][:], ps, b2_col[:, t:t + 1], x_cur[t][:],
                            mybir.AluOpType.add, mybir.AluOpType.add)
                proj_gemm(ff2T_d.ap()[l], ht, D, ffw, "fw", 10, ev_f, pbufs=4)
                do_ln(2, l)

            fillp_ctx.__exit__(None, None, None)

            # ---- final projection
            ob_col = None if skip_yb else load_bias_col(
                outb_d.ap(), VOCAB // 128, "ob")

            def ev_y(n0, ps):
                y = tmpp.tile([128, TH], F32, name=nm("y"), tag="tmp")
                if ob_col is None:
                    nc.vector.tensor_copy(y[:], ps)
                else:
                    nc.vector.tensor_scalar_add(
                        y[:], ps, ob_col[:, n0 // 128:n0 // 128 + 1])
                nc.sync.dma_start(out=yT_d.ap()[n0:n0 + 128, :], in_=y[:])
            proj_gemm(outT_d.ap(), xb16, VOCAB, ffw, "fw", 10, ev_y, pbufs=3)

    nc.compile()
    return nc


# ---------------------------------------------------------------- wrapper

def prep_in_maps(inputs):
    f32 = lambda a: np.ascontiguousarray(np.asarray(a, dtype=np.float32))
    embed = host_embed(
        np.asarray(inputs["text"]), np.asarray(inputs["audio"]),
        np.asarray(inputs["enrolled_audio"]),
        np.asarray(inputs["text_len_batch"]),
        np.asarray(inputs["audio_len_batch"]),
        f32(inputs["text_emb"]), f32(inputs["audio_emb"]))
    embT = np.ascontiguousarray(embed.transpose(0, 2, 1))   # [B, D, TPAD]

    bf = lambda a: np.ascontiguousarray(a.astype(np.float16))
    tr = lambda a: np.ascontiguousarray(
        np.asarray(a, dtype=np.float32).transpose(0, 2, 1))
    sa_inT = tr(inputs["sa_in_w"])      # [L, D, 3D]
    ca_inT = tr(inputs["ca_in_w"])
    sa_inT[:, :, :D] *= 0.125           # fold 1/sqrt(hd) into Q
    ca_inT[:, :, :D] *= 0.125
    sa_inb = f32(inputs["sa_in_b"]).copy()
    ca_inb = f32(inputs["ca_in_b"]).copy()
    sa_inb[:, :D] *= 0.125
    ca_inb[:, :D] *= 0.125

    lnwb16 = np.stack([
        np.stack([np.stack([f32(inputs[f"ln{i}_w"])[l],
                            f32(inputs[f"ln{i}_b"])[l]]) for l in range(L)])
        for i in (1, 2, 3)]).astype(np.float16)  # [3, L, 2, D]

    shared = dict(
        kval6=host_kvalid6(),
        oodc=np.full((128, 1), 1.0 / D, np.float32),
        oodc16=np.full((128, 1), 1.0 / D, np.float16),
        ones_r128=np.ones((1, 128), np.float32),
        onesb_r128=np.ones((1, 128), np.float16),
        negb_r448=np.full((1, TH), -1.0, np.float16),
        k2sel=np.concatenate([
            np.concatenate([np.ones((1, 64)), np.zeros((1, 64))], 1),
            np.concatenate([np.zeros((1, 64)), np.ones((1, 64))], 1),
        ]).astype(np.float16),
        sa_inT=bf(sa_inT), sa_outT=bf(tr(inputs["sa_out_w"])),
        ca_inT=bf(ca_inT), ca_outT=bf(tr(inputs["ca_out_w"])),
        ff1T=bf(tr(inputs["ff1_w"])), ff2T=bf(tr(inputs["ff2_w"])),
        outT=bf(np.ascontiguousarray(f32(inputs["out_w"]).T)),
        sa_inb=sa_inb, sa_outb=f32(inputs["sa_out_b"]),
        ca_inb=ca_inb, ca_outb=f32(inputs["ca_out_b"]),
        ff1b=f32(inputs["ff1_b"]), ff2b=f32(inputs["ff2_b"]),
        outb=f32(inputs["out_b"]),
        lnwb16=lnwb16,
        lnwc=np.concatenate([
            f32(inputs[f"ln{i}_w"])[l].reshape(ND, 128).T
            for i in (1, 2, 3) for l in range(L)], axis=1),
    )
    in_maps = []
    for c in range(8):
        bb, hh = c // 2, c % 2
        m = dict(shared)
        m["xT0"] = np.ascontiguousarray(embT[bb][:, hh * TH:(hh + 1) * TH])
        m["memT"] = bf(embT[bb])
        m["maskT"] = np.maximum(host_masks(hh), -30000).astype(np.float16)
        in_maps.append(m)
    return in_maps


_NC_CACHE = {}


def run(inputs, n_layers=L, trace=False):
    z = lambda a: not np.any(np.asarray(a))
    skip_bv = z(np.asarray(inputs["sa_in_b"])[:, 2 * D:]) and \
        z(np.asarray(inputs["ca_in_b"])[:, 2 * D:])
    skip_inb = z(np.asarray(inputs["sa_in_b"])[:, :2 * D]) and \
        z(np.asarray(inputs["ca_in_b"])[:, :2 * D])
    skip_outb = z(inputs["sa_out_b"]) and z(inputs["ca_out_b"])
    skip_ffb = z(inputs["ff1_b"]) and z(inputs["ff2_b"])
    skip_yb = z(inputs["out_b"])
    key = (n_layers, skip_bv, skip_inb, skip_outb, skip_ffb, skip_yb)
    if key not in _NC_CACHE:
        _NC_CACHE[key] = build_kernel(n_layers, skip_bv, skip_inb,
                                      skip_outb, skip_ffb, skip_yb)
    nc = _NC_CACHE[key]
    in_maps = prep_in_maps(inputs)
    res = bass_utils.run_bass_kernel_spmd(
        nc, in_maps, core_ids=list(range(8)), trace=trace)
    out = np.zeros((B, SEQ, VOCAB), dtype=np.float32)
    for c in range(8):
        bb, hh = c // 2, c % 2
        cols = TH if hh == 0 else SEQ - TH
        out[bb, hh * TH:hh * TH + cols, :] = \
            res.results[c]["yT"][:, :cols].T
    return out, res


def kernel(**inputs):
    out, _ = run(inputs)
    return out


# revision 19
# speedup vs baseline: 1.0396x; 1.0396x over previous
"""Trainium2 Bass kernel for nn_AutoRegressive_12128987644588.

6-layer post-norm transformer decoder (self-attn w/ prefix-causal mask,
cross-attn to packed embeddings, FFN), B=4, seq 865 (pad 896), D=1024,
16 heads x 64, FF=4096, final proj to 1024.

Sharding: 8 cores = 4 batches x 2 sequence halves of 448 tokens.
Per layer the two cores of a batch AllGather their x^T halves (the only
collective); K/V projections are computed over the full sequence on both
cores (duplicate compute, no other comm). Activations live transposed
[feature, token] in SBUF so every GEMM is natural (lhsT = W^T chunk,
rhs = x^T chunk) and all out-feature biases are per-partition. x tiles
are updated in place (residual adds and LayerNorm write back).

v2 performance structure (vs the v1 baseline):
- Attention exps are batched: scores for 2 key tiles of a head land in
  one 2-bank PSUM tile (slices at 0/512) and a single wide ACTIVATE
  covers both, amortizing the ~352-cycle ACT pipeline fill.
- No ACT table switches: LayerNorm rstd = exp(-0.5*ln(var+eps)) so the
  whole kernel lives in the natural_log_exp set (exp/ln/relu/square).
- All small stationary operands (ones rows, k2sel, LN w/b rows) are
  fp16 so their matmuls are single-pass (f32 stationaries run 2-pass
  HIGH/LOW).
- memT (cross-attn memory) is loaded to SBUF once and kept resident.
- Attention/FFN weights use separate SBUF pools so next-phase weight
  DMAs prefetch while the other phase's weights cycle.
- CA K/V projection is emitted with a 1-bank PSUM pool and late
  priority so the Tile scheduler drops its matmuls into the PE gaps of
  the (ACT-bound) self-attention phase and the AllGather window.
- Invalid-key masking for CA uses zeroed aug-ones rows in V (no exp
  bias), structurally zero biases are skipped (build flags).

Embedding gather/pack/positional encodings are integer-indexed data
staging done on host; all FLOPs run on device.
"""
import numpy as np

import concourse.bass as bass
import concourse.mybir as mybir
import concourse.tile as tile
from concourse import bacc, bass_utils

F32 = mybir.dt.float32
F32R = mybir.dt.float32r
BF16 = mybir.dt.float16  # fp16: FWL-eligible, 10-bit mantissa

B, D, H, HD, FF, L = 4, 1024, 16, 64, 4096, 6
TT, TA, ENR = 128, 512, 225
SEQ = TT + TA + ENR            # 865
TPAD = 896                     # 7 * 128
TH = 448                       # per-core half (padded)
PREFIX = TT + TA               # 640 = 5 * 128
NKT = TPAD // 128              # 7 key tiles
ND = D // 128                  # 8 feature tiles
VOCAB = 1024
EPS = 1e-5
NEG = -1e9
AF = mybir.ActivationFunctionType
WIDE_EXP = True

# The act-table-load pass maps each ACTIVATE to the first table set
# containing its function, so Exp->exp_and_others and Ln->natural_log
# alternate (a ~2.7us table DMA at every transition). Both functions
# live in natural_log_exp_and_others; restricting Exp/Ln to that one
# set makes the pass settle on it and the kernel runs with a single
# table load. Set indices are untouched, so the emitted ids stay valid.
if not getattr(bacc, "_act_tables_patched", False):
    _orig_get_act_tables = bacc.get_activation_tables

    def _patched_act_tables(arch):
        tabs = _orig_get_act_tables(arch)
        combined = "natural_log_exp_and_others"
        if combined in tabs and AF.Exp in tabs[combined]:
            for name, s in tabs.items():
                if name != combined:
                    s.discard(AF.Exp)
                    s.discard(AF.Ln)
        return tabs

    bacc.get_activation_tables = _patched_act_tables
    bacc._act_tables_patched = True


# ---------------------------------------------------------------- host side

def sinusoidal_pe(T, d):
    pos = np.arange(T, dtype=np.float32)[:, None]
    div = np.exp(np.arange(0, d, 2, dtype=np.float32) * (-np.log(10000.0) / d))
    pe = np.zeros((T, d), dtype=np.float32)
    pe[:, 0::2] = np.sin(pos * div)
    pe[:, 1::2] = np.cos(pos * div)
    return pe


def host_embed(text, audio, enrolled_audio, text_len, audio_len,
               text_emb, audio_emb):
    """Replicates reference embed+pack. Returns [B, TPAD, D] f32 (pad zeros)."""
    te = text_emb[text] + sinusoidal_pe(TT, D)[None]        # [B,TT,D]
    ae = audio_emb[audio] + sinusoidal_pe(TA, D)[None]      # [B,TA,D]
    ee = audio_emb[enrolled_audio] + sinusoidal_pe(ENR, D)[None]
    out = np.zeros((B, TPAD, D), dtype=np.float32)
    for b in range(B):
        tl, al = int(text_len[b]), int(audio_len[b])
        out[b, :tl] = te[b, :tl]
        out[b, tl:tl + al] = ae[b, :al]
        out[b, tl + al:tl + al + ENR] = ee[b]
    return out


def host_masks(half):
    """Additive mask for SA key tiles 5,6 transposed: [256, TH]."""
    k = np.arange(PREFIX, PREFIX + 256)[:, None]            # 640..895
    q = half * TH + np.arange(TH)[None, :]
    blocked = (k > q) | (k >= SEQ)
    return np.where(blocked, NEG, 0.0).astype(np.float32)


def host_kvalid6():
    k = PREFIX + 128 + np.arange(128)                       # 768..895
    return np.where(k < SEQ, 0.0, NEG).astype(np.float32)[:, None]


# ---------------------------------------------------------------- builder

def build_kernel(n_layers=L, skip_bv=False, skip_inb=False, skip_outb=False,
                 skip_ffb=False, skip_yb=False):
    nc = bacc.Bacc("TRN2", target_bir_lowering=False, debug=False,
                   num_devices=8)

    def din(name, shape, dt=F32R):
        return nc.dram_tensor(name, shape, dt, kind="ExternalInput")

    xT0_d = din("xT0", [D, TH])
    memT_d = din("memT", [D, TPAD], BF16)
    maskT_d = din("maskT", [256, TH], BF16)
    kval6_d = din("kval6", [128, 1], F32)
    oodc_d = din("oodc", [128, 1])            # 1/D column, f32r
    oodc16_d = din("oodc16", [128, 1], BF16)
    ones_r128_d = din("ones_r128", [1, 128])  # f32r ones row (bv path)
    onesb_r128_d = din("onesb_r128", [1, 128], BF16)
    k2sel_d = din("k2sel", [2, 128], BF16)
    negb_r448_d = din("negb_r448", [1, TH], BF16)

    sa_inT_d = din("sa_inT", [L, D, 3 * D], BF16)
    sa_outT_d = din("sa_outT", [L, D, D], BF16)
    ca_inT_d = din("ca_inT", [L, D, 3 * D], BF16)
    ca_outT_d = din("ca_outT", [L, D, D], BF16)
    ff1T_d = din("ff1T", [L, D, FF], BF16)
    ff2T_d = din("ff2T", [L, FF, D], BF16)
    outT_d = din("outT", [D, VOCAB], BF16)

    sa_inb_d = din("sa_inb", [L, 3 * D], F32)
    sa_outb_d = din("sa_outb", [L, D], F32)
    ca_inb_d = din("ca_inb", [L, 3 * D], F32)
    ca_outb_d = din("ca_outb", [L, D], F32)
    ff1b_d = din("ff1b", [L, FF], F32)
    ff2b_d = din("ff2b", [L, D], F32)
    outb_d = din("outb", [VOCAB], F32)
    lnwc_d = din("lnwc", [128, 3 * L * (D // 128)], F32)
    lnwb16_d = din("lnwb16", [3, L, 2, D], BF16)  # [ln idx][layer][w;b][D]

    yT_d = nc.dram_tensor("yT", [VOCAB, TH], F32, kind="ExternalOutput")

    uid = [0]

    def nm(p):
        uid[0] += 1
        return f"{p}_{uid[0]}"

    with tile.TileContext(nc) as tc:
        with (
            nc.allow_low_precision(reason="f32r/bf16 compute; tol 2e-2"),
            tc.tile_pool(name="const", bufs=1) as constp,
            tc.tile_pool(name="memp", bufs=1) as memp,
            tc.tile_pool(name="xpool", bufs=8) as xpool,
            tc.tile_pool(name="tmpp", bufs=3) as tmpp,
            tc.tile_pool(name="rows", bufs=2) as rowp,
            tc.tile_pool(name="statp", bufs=4) as statp,
            tc.tile_pool(name="biasp", bufs=10) as biasp,
            tc.tile_pool(name="attw", bufs=14) as attw,
            tc.tile_pool(name="ffw", bufs=10) as ffw,
            tc.tile_pool(name="attsb", bufs=2) as attsb,
            tc.tile_pool(name="dram", bufs=2, space="DRAM") as dramp,
        ):
            # ---- constants
            oodc = constp.tile([128, 1], F32R, name="oodc")
            oodc16 = constp.tile([128, 1], BF16, name="oodc16")
            ones_r128 = constp.tile([1, 128], F32R, name="ones_r128")
            onesb_r128 = constp.tile([1, 128], BF16, name="onesb_r128")
            kval6 = constp.tile([128, 1], F32, name="kval6")
            # wide additive mask for SA windows (5,6): slices at 0 / 512
            maskw56 = constp.tile([128, 1024], BF16, name="maskw56")
            k2sel_a = constp.tile([1, 128], BF16, name="k2sel_a")
            k2sel_b = constp.tile([1, 128], BF16, name="k2sel_b")
            lnrhs = constp.tile([2, TH], BF16, name="lnrhs")
            lnwc = constp.tile([128, 3 * L * ND], F32, name="lnwc")
            eps_tile = constp.tile([1, 1], F32, name="eps_tile")
            nc.sync.dma_start(out=oodc[:], in_=oodc_d.ap())
            nc.sync.dma_start(out=oodc16[:], in_=oodc16_d.ap())
            nc.sync.dma_start(out=ones_r128[:], in_=ones_r128_d.ap())
            nc.sync.dma_start(out=onesb_r128[:], in_=onesb_r128_d.ap())
            nc.sync.dma_start(out=kval6[:], in_=kval6_d.ap())
            nc.sync.dma_start(out=maskw56[:, 0:TH], in_=maskT_d.ap()[0:128, :])
            nc.sync.dma_start(out=maskw56[:, 512:512 + TH],
                              in_=maskT_d.ap()[128:256, :])
            nc.sync.dma_start(out=k2sel_a[:], in_=k2sel_d.ap()[0:1, :])
            nc.sync.dma_start(out=k2sel_b[:], in_=k2sel_d.ap()[1:2, :])
            nc.sync.dma_start(out=lnrhs[1:2, :], in_=negb_r448_d.ap())
            nc.sync.dma_start(out=lnwc[:], in_=lnwc_d.ap())
            nc.vector.memset(eps_tile[:], EPS)
            nc.vector.memset(maskw56[:, TH:512], 0.0)

            # ---- x tiles (f32r master) + bf16 shadows for GEMM moving ops
            x_cur = []
            xb16 = []
            for t in range(ND):
                xt = xpool.tile([128, TH], F32R, name=nm("x"), tag="x")
                nc.sync.dma_start(out=xt[:],
                                  in_=xT0_d.ap()[t * 128:(t + 1) * 128, :])
                x_cur.append(xt)
                xb = xpool.tile([128, TH], BF16, name=nm("xb"), tag="xb")
                nc.vector.tensor_copy(xb[:], xt[:])
                xb16.append(xb)

            # ---- persistent cross-attention memory (layer-invariant)
            memt = []
            for t in range(ND):
                mt = memp.tile([128, TPAD], BF16, name=nm("memt"), tag="mem",
                               bufs=8)
                nc.sync.dma_start(
                    out=mt[:], in_=memT_d.ap()[t * 128:(t + 1) * 128, :])
                memt.append(mt)

            # ---------------------------------------------------- helpers
            def load_bias_col(src_1d_ap, n, name):
                t = biasp.tile([128, n], F32, name=nm(name), tag="bcol")
                nc.sync.dma_start(
                    out=t[:], in_=src_1d_ap.rearrange("(c p) -> p c", p=128))
                return t

            def load_row(src_1d_ap, n, name):
                t = rowp.tile([1, n], F32R, name=nm(name), tag="row")
                nc.sync.dma_start(
                    out=t[:],
                    in_=src_1d_ap.rearrange("(a f) -> a f", a=1).bitcast(F32R))
                return t

            def proj_gemm(wT2d, rhs_tiles, nout, wpool, wtag, wbufs, evict,
                          fdim=TH, pbufs=3):
                """out^T[nout, fdim] = W @ rhs. evict(n0, psum) per 128 rows."""
                nk = len(rhs_tiles)
                ctx = tc.tile_pool(name=nm("gps"), bufs=pbufs, space="PSUM")
                ppool = ctx.__enter__()
                for n0 in range(0, nout, 512):
                    w = min(512, nout - n0)
                    wts = []
                    for k in range(nk):
                        wt = wpool.tile([128, w], BF16, name=nm("w"), tag=wtag,
                                        bufs=wbufs)
                        nc.sync.dma_start(
                            out=wt[:],
                            in_=wT2d[k * 128:(k + 1) * 128, n0:n0 + w])
                        wts.append(wt)
                    for m0 in range(0, w, 128):
                        ps = ppool.tile([128, fdim], F32, name=nm("pg"),
                                        tag="pg", bufs=pbufs)
                        for k in range(nk):
                            nc.tensor.matmul(
                                ps[:], wts[k][:, m0:m0 + 128],
                                rhs_tiles[k][:, :fdim],
                                start=(k == 0), stop=(k == nk - 1))
                        evict(n0 + m0, ps)
                ctx.__exit__(None, None, None)

            def layer_norm(x_tiles, w_col, w_row):
                """In-place post-norm LN over the feature (partition) dim.
                rstd = exp(-0.5*ln(var+eps)) keeps ACT in the ln/exp table
                set; stats stay in PSUM and feed DVE directly."""
                with tc.tile_pool(name=nm("lnps"), bufs=1, space="PSUM") as lps:
                    mu_ps = lps.tile([1, TH], F32, name=nm("mups"), bufs=1)
                    s2_ps = lps.tile([1, TH], F32, name=nm("s2ps"), bufs=1)
                    for t in range(ND):
                        nc.tensor.matmul(mu_ps[:], oodc[:], x_tiles[t][:],
                                         start=(t == 0), stop=(t == ND - 1))
                    for t in range(ND):
                        sq = tmpp.tile([128, TH], BF16, name=nm("sq"),
                                       tag="sq16", bufs=3)
                        nc.vector.tensor_tensor(sq[:], x_tiles[t][:],
                                                x_tiles[t][:],
                                                mybir.AluOpType.mult)
                        nc.tensor.matmul(s2_ps[:], oodc16[:], sq[:],
                                         start=(t == 0), stop=(t == ND - 1))
                    mu_sb = statp.tile([1, TH], F32, name=nm("musb"),
                                       tag="st")
                    nc.vector.tensor_copy(mu_sb[:], mu_ps[:])
                    musq = statp.tile([1, TH], F32, name=nm("musq"),
                                      tag="st")
                    nc.vector.tensor_tensor(musq[:], mu_sb[:], mu_sb[:],
                                            mybir.AluOpType.mult)
                    var = statp.tile([1, TH], F32, name=nm("var"), tag="st")
                    nc.vector.tensor_tensor(var[:], s2_ps[:], musq[:],
                                            mybir.AluOpType.subtract)
                    lnv = statp.tile([1, TH], F32, name=nm("lnv"), tag="st")
                    nc.scalar.activation(lnv[:], var[:], AF.Ln,
                                         bias=eps_tile[:])
                    rstd = statp.tile([1, TH], BF16, name=nm("rstd"),
                                      tag="st")
                    nc.scalar.activation(rstd[:], lnv[:], AF.Exp, scale=-0.5)
                    rsb_ps = lps.tile([128, TH], F32, name=nm("rsb"), bufs=1)
                    nc.tensor.matmul(rsb_ps[:], onesb_r128[:], rstd[:],
                                     start=True, stop=True)
                    nc.vector.tensor_tensor(lnrhs[0:1, :], mu_sb[:], rstd[:],
                                            mybir.AluOpType.mult)
                    for t in range(ND):
                        aux = lps.tile([128, TH], F32, name=nm("aux"),
                                       tag="lnaux", bufs=2)
                        nc.tensor.matmul(aux[:],
                                         w_row[:, t * 128:(t + 1) * 128],
                                         lnrhs[:], start=True, stop=True)
                        t1 = tmpp.tile([128, TH], F32R, name=nm("t1"),
                                       tag="tmp")
                        nc.vector.tensor_tensor(t1[:], x_tiles[t][:],
                                                rsb_ps[:],
                                                mybir.AluOpType.mult)
                        nc.vector.scalar_tensor_tensor(
                            x_tiles[t][:], t1[:], w_col[:, t:t + 1], aux[:],
                            mybir.AluOpType.mult, mybir.AluOpType.subtract)
                        nc.scalar.copy(xb16[t][:], x_tiles[t][:])

            def attention(q_tiles, kt_tiles, vaug_tiles, masked, kval):
                """Returns attnT tiles (8 x [128, TH]).
                Heads processed singly; per head the 7 key tiles are
                grouped into windows of 2 sharing one 2-bank PSUM tile so
                a single wide ACTIVATE computes both exps. SA groups the
                masked tiles (5,6) together and adds maskw56 in one wide
                DVE op. PSUM: swide 2x2 + o_ps 2 + rps 1 + filler 1 = 8."""
                at = [attsb.tile([128, TH], BF16, name=nm("at"), tag="attnT",
                                 bufs=8) for _ in range(ND)]
                if masked:
                    windows = [(0, 1), (2, 3), (4,), (5, 6)]
                else:
                    windows = [(0, 1), (2, 3), (4, 5), (6,)]
                with (
                    tc.tile_pool(name=nm("aps"), bufs=2, space="PSUM") as sps,
                    tc.tile_pool(name=nm("ops"), bufs=2, space="PSUM") as ops,
                    tc.tile_pool(name=nm("bps"), bufs=1, space="PSUM") as bps,
                ):
                    o_ps = {}
                    for hh in range(H):
                        ti, r0 = hh // 2, (hh % 2) * 64
                        o_ps[hh] = ops.tile([65, TH], F32, name=nm("ops"),
                                            tag="po", bufs=2)
                        for win in windows:
                            sw = sps.tile([128, 1024], F32, name=nm("sw"),
                                          tag="sw", bufs=2)
                            if WIDE_EXP and len(win) == 2:
                                nc.vector.memset(sw[:, TH:512], 0.0)
                            for j, t in enumerate(win):
                                nc.tensor.matmul(
                                    sw[:, j * 512:j * 512 + TH],
                                    kt_tiles[ti][r0:r0 + 64,
                                                 t * 128:(t + 1) * 128],
                                    q_tiles[ti][r0:r0 + 64, :],
                                    start=True, stop=True)
                            wd = (win[-1] - win[0]) * 512 + TH
                            pbw = attsb.tile([128, 1024], BF16, name=nm("p"),
                                             tag="pexp", bufs=3)
                            if WIDE_EXP:
                                if masked and win[0] == 5:
                                    sm = attsb.tile([128, 1024], BF16,
                                                    name=nm("sm"), tag="pexp",
                                                    bufs=3)
                                    nc.vector.tensor_tensor(
                                        sm[:, 0:wd], sw[:, 0:wd],
                                        maskw56[:, 0:wd],
                                        mybir.AluOpType.add)
                                    nc.scalar.activation(pbw[:, 0:wd],
                                                         sm[:, 0:wd], AF.Exp)
                                elif (kval is not None
                                      and win[-1] == NKT - 1):
                                    nc.scalar.activation(pbw[:, 0:wd],
                                                         sw[:, 0:wd], AF.Exp,
                                                         bias=kval[:])
                                else:
                                    nc.scalar.activation(pbw[:, 0:wd],
                                                         sw[:, 0:wd], AF.Exp)
                            else:
                                for j, t in enumerate(win):
                                    c0 = j * 512
                                    if masked and win[0] == 5:
                                        sm = attsb.tile(
                                            [128, 1024], BF16,
                                            name=nm("sm"), tag="pexp",
                                            bufs=3)
                                        nc.vector.tensor_tensor(
                                            sm[:, c0:c0 + TH],
                                            sw[:, c0:c0 + TH],
                                            maskw56[:, c0:c0 + TH],
                                            mybir.AluOpType.add)
                                        nc.scalar.activation(
                                            pbw[:, c0:c0 + TH],
                                            sm[:, c0:c0 + TH], AF.Exp)
                                    elif (kval is not None
                                          and t == NKT - 1):
                                        nc.scalar.activation(
                                            pbw[:, c0:c0 + TH],
                                            sw[:, c0:c0 + TH], AF.Exp,
                                            bias=kval[:])
                                    else:
                                        nc.scalar.activation(
                                            pbw[:, c0:c0 + TH],
                                            sw[:, c0:c0 + TH], AF.Exp)
                            for j, t in enumerate(win):
                                nc.tensor.matmul(
                                    o_ps[hh][:],
                                    vaug_tiles[t][:].rearrange(
                                        "p (h e) -> p h e", e=65)[:, hh, :],
                                    pbw[:, j * 512:j * 512 + TH],
                                    start=(t == 0), stop=(t == NKT - 1))
                        if hh % 2 == 1:
                            hp = hh - 1
                            r_ps = bps.tile([128, TH], F32, name=nm("rps"),
                                            tag="pb", bufs=1)
                            for h2 in (hp, hp + 1):
                                den = statp.tile([1, TH], BF16,
                                                 name=nm("den"), tag="st")
                                nc.vector.tensor_copy(den[:],
                                                      o_ps[h2][64:65, :])
                                nc.tensor.matmul(r_ps[:],
                                                 (k2sel_a if h2 % 2 == 0
                                                  else k2sel_b)[:],
                                                 den[:],
                                                 start=(h2 % 2 == 0),
                                                 stop=(h2 % 2 == 1))
                            rb = tmpp.tile([128, TH], F32, name=nm("rb"),
                                           tag="rb", bufs=2)
                            nc.vector.reciprocal_approx_fast(out=rb[:],
                                                             in_=r_ps[:])
                            for h2 in (hp, hp + 1):
                                ti2, r2 = h2 // 2, (h2 % 2) * 64
                                nc.vector.tensor_tensor(
                                    at[ti2][r2:r2 + 64, :],
                                    o_ps[h2][0:64, :],
                                    rb[r2:r2 + 64, :],
                                    mybir.AluOpType.mult)
                return at

            def kv_gemm(tag, inT2d, inb1d, src_tiles, vones_t6, psum_pool,
                        pbufs):
                """K^T tiles [8 x (128, TPAD)] + V_aug [7 x (128, H*65)].
                K weights are loaded once and reused for both halves."""
                kt = [attsb.tile([128, TPAD], BF16, name=nm("kt"),
                                 tag=tag + "k", bufs=8) for _ in range(ND)]
                bk_col = None if skip_inb else load_bias_col(
                    inb1d[D:2 * D], ND, "bk")
                for c0 in (0, 512):
                    wts = []
                    for k in range(ND):
                        wt = attw.tile([128, 512], BF16, name=nm("wk"),
                                       tag="aw", bufs=14)
                        nc.sync.dma_start(
                            out=wt[:],
                            in_=inT2d[k * 128:(k + 1) * 128,
                                      D + c0:D + c0 + 512])
                        wts.append(wt)
                    for f0 in (0, TH):
                        for m0 in range(0, 512, 128):
                            ps = psum_pool.tile([128, TH], F32, name=nm("pk"),
                                                tag="pg", bufs=pbufs)
                            for k in range(ND):
                                nc.tensor.matmul(
                                    ps[:], wts[k][:, m0:m0 + 128],
                                    src_tiles[k][:, f0:f0 + TH],
                                    start=(k == 0), stop=(k == ND - 1))
                            ti = (c0 + m0) // 128
                            if bk_col is None:
                                nc.vector.tensor_copy(
                                    kt[ti][:, f0:f0 + TH], ps[:])
                            else:
                                nc.vector.tensor_scalar_add(
                                    kt[ti][:, f0:f0 + TH], ps,
                                    bk_col[:, ti:ti + 1])
                va = [attsb.tile([128, H * 65], BF16, name=nm("va"),
                                 tag=tag + "v", bufs=NKT) for _ in range(NKT)]
                bv_row = None if skip_bv else load_row(
                    inb1d[2 * D:3 * D], D, "bv")
                for t in range(NKT):
                    aug = va[t][:].rearrange("p (h e) -> p h e",
                                             e=65)[:, :, 64:65]
                    if t == NKT - 1 and vones_t6:
                        nc.vector.memset(aug, 0.0)
                        nc.vector.memset(aug[0:SEQ - PREFIX - 128], 1.0)
                    else:
                        nc.vector.memset(aug, 1.0)
                for c0 in (0, 512):
                    wts = []
                    for k in range(ND):
                        wt = attw.tile([128, 512], BF16, name=nm("wv"),
                                       tag="aw", bufs=14)
                        nc.sync.dma_start(
                            out=wt[:],
                            in_=inT2d[k * 128:(k + 1) * 128,
                                      2 * D + c0:2 * D + c0 + 512])
                        wts.append(wt)
                    for t in range(NKT):
                        ps = psum_pool.tile([128, 512], F32, name=nm("pv"),
                                            tag="pg", bufs=pbufs)
                        for k in range(ND):
                            nc.tensor.matmul(
                                ps[:],
                                src_tiles[k][:, t * 128:(t + 1) * 128],
                                wts[k][:], start=(k == 0),
                                stop=(bv_row is None and k == ND - 1))
                        if bv_row is not None:
                            nc.tensor.matmul(ps[:], ones_r128[:, :128],
                                             bv_row[:, c0:c0 + 512],
                                             start=False, stop=True)
                        nc.vector.tensor_copy(
                            va[t][:].rearrange("p (h e) -> p h e", e=65)
                            [:, c0 // 64:c0 // 64 + 8, 0:64],
                            ps[:].rearrange("p (h e) -> p h e", e=64))
                return kt, va

            def qproj(inT2d, inb1d):
                q_t = [attsb.tile([128, TH], BF16, name=nm("q"), tag="q",
                                  bufs=8) for _ in range(ND)]
                bq_col = None if skip_inb else load_bias_col(
                    inb1d[0:D], ND, "bq")

                def ev_q(n0, ps):
                    ti = n0 // 128
                    if bq_col is None:
                        nc.vector.tensor_copy(q_t[ti][:], ps)
                    else:
                        nc.vector.tensor_scalar_add(
                            q_t[ti][:], ps, bq_col[:, ti:ti + 1])
                proj_gemm(inT2d[:, 0:D], xb16, D, attw, "aw", 14, ev_q, pbufs=2)
                return q_t

            def out_proj(wT2d, b1d, at):
                bo_col = None if skip_outb else load_bias_col(b1d, ND, "bo")

                def ev_o(n0, ps):
                    t = n0 // 128
                    if bo_col is None:
                        nc.vector.tensor_tensor(
                            x_cur[t][:], ps, x_cur[t][:],
                            mybir.AluOpType.add)
                    else:
                        nc.vector.scalar_tensor_tensor(
                            x_cur[t][:], ps, bo_col[:, t:t + 1], x_cur[t][:],
                            mybir.AluOpType.add, mybir.AluOpType.add)
                proj_gemm(wT2d, at, D, attw, "aw", 14, ev_o, pbufs=2)

            def do_ln(idx, l):
                lwb = rowp.tile([2, D], BF16, name=nm(f"ln{idx}wb"),
                                tag="rowb")
                nc.sync.dma_start(out=lwb[:], in_=lnwb16_d.ap()[idx, l])
                b0 = (idx * L + l) * ND
                layer_norm(x_cur, lnwc[:, b0:b0 + ND], lwb)

            # ---------------------------------------------------- layers
            for l in range(n_layers):
                ag_in = dramp.tile([D, TH], BF16, name=nm("agin"), tag="agi")
                ag_out = dramp.tile([2 * D, TH], BF16, name=nm("agout"),
                                    tag="ago")
                for t in range(ND):
                    nc.sync.dma_start(
                        out=ag_in[t * 128:(t + 1) * 128, :], in_=xb16[t][:])
                nc.gpsimd.collective_compute(
                    "AllGather", mybir.AluOpType.bypass,
                    replica_groups=[[0, 1], [2, 3], [4, 5], [6, 7]],
                    ins=[ag_in[:].opt()], outs=[ag_out[:].opt()])

                fillp_ctx = tc.tile_pool(name=nm("fillp"), bufs=1,
                                         space="PSUM")
                fillp = fillp_ctx.__enter__()
                q_sa = qproj(sa_inT_d.ap()[l], sa_inb_d.ap()[l])

                # SA side (needs the gathered full sequence)
                xfull = [attsb.tile([128, TPAD], BF16, name=nm("xf"),
                                    tag="xfull", bufs=8) for _ in range(ND)]
                for t in range(ND):
                    nc.sync.dma_start(
                        out=xfull[t][:, 0:TH],
                        in_=ag_out[t * 128:(t + 1) * 128, :])
                    nc.sync.dma_start(
                        out=xfull[t][:, TH:TPAD],
                        in_=ag_out[D + t * 128:D + (t + 1) * 128, :])
                with tc.tile_pool(name=nm("sakvp"), bufs=2,
                                  space="PSUM") as sakvp:
                    kt_sa, va_sa = kv_gemm("sa", sa_inT_d.ap()[l],
                                           sa_inb_d.ap()[l], xfull, False,
                                           sakvp, 2)
                at = attention(q_sa, kt_sa, va_sa, True, None)
                kt_ca, va_ca = kv_gemm("ca", ca_inT_d.ap()[l],
                                       ca_inb_d.ap()[l], memt, skip_bv,
                                       fillp, 1)
                fillp_ctx.__exit__(None, None, None)
                out_proj(sa_outT_d.ap()[l], sa_outb_d.ap()[l], at)
                do_ln(0, l)

                q_ca = qproj(ca_inT_d.ap()[l], ca_inb_d.ap()[l])
                at = attention(q_ca, kt_ca, va_ca, False,
                               None if skip_bv else kval6)
                out_proj(ca_outT_d.ap()[l], ca_outb_d.ap()[l], at)
                do_ln(1, l)

                # ================= FFN =================
                ht = [attsb.tile([128, TH], BF16, name=nm("h"), tag="h",
                                 bufs=FF // 128) for _ in range(FF // 128)]
                b1_col = None if skip_ffb else load_bias_col(
                    ff1b_d.ap()[l], FF // 128, "b1")

                def ev_h(n0, ps, ht=ht, b1_col=b1_col):
                    t = n0 // 128
                    if b1_col is None:
                        nc.scalar.activation(ht[t][:], ps, AF.Relu)
                    else:
                        nc.scalar.activation(ht[t][:], ps, AF.Relu,
                                             bias=b1_col[:, t:t + 1])
                proj_gemm(ff1T_d.ap()[l], xb16, FF, ffw, "fw", 10, ev_h,
                          pbufs=4)

                b2_col = None if skip_ffb else load_bias_col(
                    ff2b_d.ap()[l], ND, "b2")

                def ev_f(n0, ps, b2_col=b2_col):
                    t = n0 // 128
                    if b2_col is None:
                        nc.vector.tensor_tensor(
                            x_cur[t][:], ps, x_cur[t][:],
                            mybir.AluOpType.add)
                    else:
                        nc.vector.scalar_tensor_tensor(
                            x_cur[t][:], ps, b2_col[:, t:t + 1], x_cur[t][:],
                            mybir.AluOpType.add, mybir.AluOpType.add)
                proj_gemm(ff2T_d.ap()[l], ht, D, ffw, "fw", 10, ev_f, pbufs=4)
                do_ln(2, l)

            # ---- final projection
            ob_col = None if skip_yb else load_bias_col(
                outb_d.ap(), VOCAB // 128, "ob")

            def ev_y(n0, ps):
                y = tmpp.tile([128, TH], F32, name=nm("y"), tag="tmp")
                if ob_col is None:
                    nc.vector.tensor_copy(y[:], ps)
                else:
                    nc.vector.tensor_scalar_add(
                        y[:], ps, ob_col[:, n0 // 128:n0 // 128 + 1])
                nc.sync.dma_start(out=yT_d.ap()[n0:n0 + 128, :], in_=y[:])
            proj_gemm(outT_d.ap(), xb16, VOCAB, ffw, "fw", 10, ev_y, pbufs=3)

    nc.compile()
    return nc


# ---------------------------------------------------------------- wrapper

def prep_in_maps(inputs):
    f32 = lambda a: np.ascontiguousarray(np.asarray(a, dtype=np.float32))
    embed = host_embed(
        np.asarray(inputs["text"]), np.asarray(inputs["audio"]),
        np.asarray(inputs["enrolled_audio"]),
        np.asarray(inputs["text_len_batch"]),
        np.asarray(inputs["audio_len_batch"]),
        f32(inputs["text_emb"]), f32(inputs["audio_emb"]))
    embT = np.ascontiguousarray(embed.transpose(0, 2, 1))   # [B, D, TPAD]

    bf = lambda a: np.ascontiguousarray(a.astype(np.float16))
    tr = lambda a: np.ascontiguousarray(
        np.asarray(a, dtype=np.float32).transpose(0, 2, 1))
    sa_inT = tr(inputs["sa_in_w"])      # [L, D, 3D]
    ca_inT = tr(inputs["ca_in_w"])
    sa_inT[:, :, :D] *= 0.125           # fold 1/sqrt(hd) into Q
    ca_inT[:, :, :D] *= 0.125
    sa_inb = f32(inputs["sa_in_b"]).copy()
    ca_inb = f32(inputs["ca_in_b"]).copy()
    sa_inb[:, :D] *= 0.125
    ca_inb[:, :D] *= 0.125

    lnwb16 = np.stack([
        np.stack([np.stack([f32(inputs[f"ln{i}_w"])[l],
                            f32(inputs[f"ln{i}_b"])[l]]) for l in range(L)])
        for i in (1, 2, 3)]).astype(np.float16)  # [3, L, 2, D]

    shared = dict(
        kval6=host_kvalid6(),
        oodc=np.full((128, 1), 1.0 / D, np.float32),
        oodc16=np.full((128, 1), 1.0 / D, np.float16),
        ones_r128=np.ones((1, 128), np.float32),
        onesb_r128=np.ones((1, 128), np.float16),
        negb_r448=np.full((1, TH), -1.0, np.float16),
        k2sel=np.concatenate([
            np.concatenate([np.ones((1, 64)), np.zeros((1, 64))], 1),
            np.concatenate([np.zeros((1, 64)), np.ones((1, 64))], 1),
        ]).astype(np.float16),
        sa_inT=bf(sa_inT), sa_outT=bf(tr(inputs["sa_out_w"])),
        ca_inT=bf(ca_inT), ca_outT=bf(tr(inputs["ca_out_w"])),
        ff1T=bf(tr(inputs["ff1_w"])), ff2T=bf(tr(inputs["ff2_w"])),
        outT=bf(np.ascontiguousarray(f32(inputs["out_w"]).T)),
        sa_inb=sa_inb, sa_outb=f32(inputs["sa_out_b"]),
        ca_inb=ca_inb, ca_outb=f32(inputs["ca_out_b"]),
        ff1b=f32(inputs["ff1_b"]), ff2b=f32(inputs["ff2_b"]),
        outb=f32(inputs["out_b"]),
        lnwb16=lnwb16,
        lnwc=np.concatenate([
            f32(inputs[f"ln{i}_w"])[l].reshape(ND, 128).T
            for i in (1, 2, 3) for l in range(L)], axis=1),
    )
    in_maps = []
    for c in range(8):
        bb, hh = c // 2, c % 2
        m = dict(shared)
        m["xT0"] = np.ascontiguousarray(embT[bb][:, hh * TH:(hh + 1) * TH])
        m["memT"] = bf(embT[bb])
        m["maskT"] = np.maximum(host_masks(hh), -30000).astype(np.float16)
        in_maps.append(m)
    return in_maps


_NC_CACHE = {}


def run(inputs, n_layers=L, trace=False):
    z = lambda a: not np.any(np.asarray(a))
    skip_bv = z(np.asarray(inputs["sa_in_b"])[:, 2 * D:]) and \
        z(np.asarray(inputs["ca_in_b"])[:, 2 * D:])
    skip_inb = z(np.asarray(inputs["sa_in_b"])[:, :2 * D]) and \
        z(np.asarray(inputs["ca_in_b"])[:, :2 * D])
    skip_outb = z(inputs["sa_out_b"]) and z(inputs["ca_out_b"])
    skip_ffb = z(inputs["ff1_b"]) and z(inputs["ff2_b"])
    skip_yb = z(inputs["out_b"])
    key = (n_layers, skip_bv, skip_inb, skip_outb, skip_ffb, skip_yb)
    if key not in _NC_CACHE:
        _NC_CACHE[key] = build_kernel(n_layers, skip_bv, skip_inb,
                                      skip_outb, skip_ffb, skip_yb)
    nc = _NC_CACHE[key]
    in_maps = prep_in_maps(inputs)
    res = bass_utils.run_bass_kernel_spmd(
        nc, in_maps, core_ids=list(range(8)), trace=trace)
    out = np.zeros((B, SEQ, VOCAB), dtype=np.float32)
    for c in range(8):
        bb, hh = c // 2, c % 2
        cols = TH if hh == 0 else SEQ - TH
        out[bb, hh * TH:hh * TH + cols, :] = \
            res.results[c]["yT"][:, :cols].T
    return out, res


def kernel(**inputs):
    out, _ = run(inputs)
    return out
